# revision 1
# baseline (speedup 1.0000x reference)
# Trainium2 Bass kernel for relative-position causal attention
# (Transformer-XL style: logits = q·k + q·table[n-m], causal softmax, AV, out-proj).
#
# Sharding: tensor-parallel over heads — 16 heads / 8 cores = 2 heads per core.
# Each core computes its heads' projections, attention, and a partial output
# projection [N,B,D]; the host sums the 8 partials.
#
# Key device-side trick: position logits are computed in *diagonal* layout
# (T[ni, j] = q[nb+ni] · table[j], a plain matmul since the table index is the
# diagonal n-m), then converted to row layout with a DMA "shear" through a DRAM
# scratch strip: partition ni reads flat offset ni*(W-1) + c + mj, which is
# exactly the per-partition-shifted gather no on-chip engine can do.
#
# Precision: logit-affecting matmuls (q/k/table projections, content and
# position logits) run in true fp32 on the PE (fp32r is ~13-bit and flips
# argmaxes; the un-scaled logits here reach +-83000 with top-2 gaps as small as
# 0.014). The value path (v projection, softmax weights, AV, out projection)
# runs in fp16, which perturbs the output by ~1e-4 relative — far below the
# fp32 reference's own ~1e-2 noise floor.

from contextlib import ExitStack

import numpy as np

N = 2048
M = 2048
B = 2
D = 1024
H = 16
DQK = 64
DV = 64
NCORES = 8
HPC = H // NCORES  # heads per core = 2
NZ = N * B
KT = D // 128  # 8 contraction tiles
TW = 2048  # table width (diagonals 0..2047)

_cache = {}


def _sincos_rev():
    """sincos basis for diagonals d=0..2047, column-reversed, transposed to
    [D, TW] so sctr[:, jr] = sincos(d=2047-jr).  Computed with jax on CPU to
    match the reference's fp32 rounding of inv_freq/phases/sin bitwise."""
    try:
        import jax
        import jax.numpy as jnp

        cpu = jax.devices("cpu")[0]
        with jax.default_device(cpu):
            r = jnp.arange(0.0, float(TW), dtype=jnp.float32)
            inv_freq = 1.0 / (
                10000.0 ** (jnp.arange(0.0, D, 2.0, dtype=jnp.float32) / D)
            )
            phases = r[:, None] * inv_freq[None, :]
            sincos = jnp.concatenate([jnp.sin(phases), jnp.cos(phases)], axis=-1)
            sc = np.asarray(sincos)  # [TW, D]
    except Exception:
        r = np.arange(0.0, float(TW), dtype=np.float32)
        inv_freq = (
            1.0
            / (10000.0 ** (np.arange(0.0, D, 2.0, dtype=np.float32) / np.float32(D)))
        ).astype(np.float32)
        phases = (r[:, None] * inv_freq[None, :]).astype(np.float32)
        sc = np.concatenate(
            [np.sin(phases, dtype=np.float32), np.cos(phases, dtype=np.float32)],
            axis=-1,
        )
    return np.ascontiguousarray(sc[::-1].T.astype(np.float32))  # [D, TW]


def _build(loop=1, dbg=False):
    import concourse.bacc as bacc
    import concourse.mybir as mybir
    import concourse.tile as tile
    from concourse.bass import AP
    from concourse.masks import make_identity
    from concourse.tile_rust import add_dep_helper

    f32 = mybir.dt.float32
    f16 = mybir.dt.float16
    AX = mybir.AxisListType.X
    MAX = mybir.AluOpType.max
    EXP = mybir.ActivationFunctionType.Exp

    nc = bacc.Bacc("TRN2", target_bir_lowering=False, debug=False, num_devices=NCORES)

    xqT = nc.dram_tensor("xqT", [D, NZ], f32, kind="ExternalInput")
    xkvT = nc.dram_tensor("xkvT", [D, NZ], f32, kind="ExternalInput")
    xkv16 = nc.dram_tensor("xkv16", [D, NZ], f16, kind="ExternalInput")
    sctr = nc.dram_tensor("sctr", [D, TW], f32, kind="ExternalInput")
    wqT = nc.dram_tensor("wqT", [D, 128], f32, kind="ExternalInput")
    wkT = nc.dram_tensor("wkT", [D, 128], f32, kind="ExternalInput")
    wvT = nc.dram_tensor("wvT", [D, 128], f16, kind="ExternalInput")
    wpT = nc.dram_tensor("wpT", [D, 128], f32, kind="ExternalInput")
    woT = nc.dram_tensor("woT", [128, D], f16, kind="ExternalInput")
    outT = nc.dram_tensor("outT", [B, D, N], f32, kind="ExternalOutput")
    dbg_t = {}
    if dbg:
        for nm, shp, dt in [
            ("d_qT", [128, NZ], f32),
            ("d_kT", [128, NZ], f32),
            ("d_tabT", [128, TW], f32),
            ("d_V", [128, B * 16 * 128], f16),
            ("d_L15", [128, 2048], f32),
            ("d_P15", [128, 2048], f16),
            ("d_attn16", [128, NZ], f16),
        ]:
            dbg_t[nm] = nc.dram_tensor(nm, shp, dt, kind="ExternalOutput")

    # DRAM scratch strips for the diagonal->row shear, one per (z, h, i).
    scr = {}
    for z in range(B):
        for h in range(HPC):
            for i in range(16):
                W = 128 * (i + 1)
                scr[(z, h, i)] = nc.dram_tensor(
                    f"scr_{z}_{h}_{i}", [128 * W + 512], f32, kind="Internal"
                )

    with tile.TileContext(nc) as tc:
        with ExitStack() as ctx:
            wpool = ctx.enter_context(tc.tile_pool(name="wpool", bufs=1))
            big = ctx.enter_context(tc.tile_pool(name="big", bufs=1))
            xp = ctx.enter_context(tc.tile_pool(name="xp", bufs=2))
            work = ctx.enter_context(tc.tile_pool(name="work", bufs=3))
            lpool = ctx.enter_context(tc.tile_pool(name="lpool", bufs=2))
            tpool = ctx.enter_context(tc.tile_pool(name="tpool", bufs=2))

            # ---- constants / weights ----
            ident16 = wpool.tile([128, 128], f16)
            make_identity(nc, ident16[:])
            ident32 = wpool.tile([128, 128], f32)
            make_identity(nc, ident32[:])
            wq_sb = wpool.tile([128, KT, 128], f32)
            wk_sb = wpool.tile([128, KT, 128], f32)
            wv_sb = wpool.tile([128, KT, 128], f16)
            wp_sb = wpool.tile([128, KT, 128], f32)
            wo_sb = wpool.tile([128, D], f16)
            nc.sync.dma_start(wq_sb[:], wqT.ap().rearrange("(t p) m -> p t m", p=128))
            nc.sync.dma_start(wk_sb[:], wkT.ap().rearrange("(t p) m -> p t m", p=128))
            nc.sync.dma_start(wv_sb[:], wvT.ap().rearrange("(t p) m -> p t m", p=128))
            nc.sync.dma_start(wp_sb[:], wpT.ap().rearrange("(t p) m -> p t m", p=128))
            nc.sync.dma_start(wo_sb[:], woT.ap())

            # ---- persistent activations ----
            qT_all = big.tile([128, NZ], f32)  # [hd, z*N+n]
            kT_all = big.tile([128, NZ], f32)
            tabT = big.tile([128, TW], f32)  # reversed diagonal table, [hd, jr]
            V_all = big.tile([128, B * 16, 128], f16)  # [m-part, z*16+mb, hv]
            attn16 = big.tile([128, NZ], f16)  # [hv, z*N+n], normalized

            def phase_proj():
                with tc.tile_pool(name="pp", bufs=1, space="PSUM") as pp:

                    def project(x_dram, w_sb, out_sb, ncols, dt_x):
                        nchunks = ncols // 512
                        psums = [
                            pp.tile([128, 512], f32, tag=f"pj{c}", name=f"pj{c}")
                            for c in range(nchunks)
                        ]
                        for t in range(KT):
                            x_t = xp.tile([128, ncols], dt_x, tag=f"x{dt_x}")
                            nc.sync.dma_start(
                                x_t[:],
                                x_dram.ap().rearrange("(t p) n -> p t n", p=128)[
                                    :, t, :
                                ],
                            )
                            for c in range(nchunks):
                                nc.tensor.matmul(
                                    psums[c][:],
                                    w_sb[:, t, :],
                                    x_t[:, 512 * c : 512 * (c + 1)],
                                    start=(t == 0),
                                    stop=(t == KT - 1),
                                )
                        for c in range(nchunks):
                            nc.scalar.copy(
                                out_sb[:, 512 * c : 512 * (c + 1)], psums[c][:]
                            )

                    project(xqT, wq_sb, qT_all, NZ, f32)
                    project(xkvT, wk_sb, kT_all, NZ, f32)
                    project(sctr, wp_sb, tabT, TW, f32)
                    vT16 = xp.tile([128, NZ], f16, tag=f"x{f16}")
                    project(xkv16, wv_sb, vT16, NZ, f16)

                    # V tiles [m, hv] via PE transposes of vT16
                    for z in range(B):
                        for mb in range(16):
                            vp = pp.tile([128, 128], f16, tag="pj0")
                            nc.tensor.transpose(
                                vp[:],
                                vT16[:, z * N + 128 * mb : z * N + 128 * (mb + 1)],
                                ident16[:],
                            )
                            nc.vector.tensor_copy(V_all[:, z * 16 + mb, :], vp[:])

            def phase_attn(pa):
                for z in range(B):
                    for h in range(HPC):
                        hs = slice(64 * h, 64 * (h + 1))
                        for i in range(16):
                            nb = 128 * i
                            W = nb + 128
                            chunks = i // 4 + 1
                            c_diag = i // 4
                            q_blk = qT_all[hs, z * N + nb : z * N + nb + 128]

                            # --- position logits in diagonal layout (reversed) ---
                            tT_sb = tpool.tile([128, 2048], f32, tag="tT")
                            nbc = (W + 511) // 512
                            for bc in range(nbc):
                                wdt = min(512, W - 512 * bc)
                                t_ps = pa.tile([128, 512], f32, tag="tps")
                                nc.tensor.matmul(
                                    t_ps[:, :wdt],
                                    q_blk,
                                    tabT[hs, TW - W + 512 * bc : TW - W + 512 * bc + wdt],
                                    start=True,
                                    stop=True,
                                )
                                nc.scalar.copy(
                                    tT_sb[:, 512 * bc : 512 * bc + wdt], t_ps[:, :wdt]
                                )
                            s = scr[(z, h, i)]
                            w_inst = nc.sync.dma_start(
                                AP(s, 0, [[W, 128], [1, W]]), tT_sb[:, :W]
                            )

                            # --- logits chunks: content matmul + sheared position ---
                            L_sb = lpool.tile([128, 2048], f32, tag="L")
                            cmax = work.tile([128, 4], f32, tag="cmax")
                            for c in range(chunks):
                                c_ps = pa.tile([128, 512], f32, tag="cps")
                                nc.tensor.matmul(
                                    c_ps[:],
                                    q_blk,
                                    kT_all[hs, z * N + 512 * c : z * N + 512 * (c + 1)],
                                    start=True,
                                    stop=True,
                                )
                                Lp = work.tile([128, 512], f32, tag="Lp")
                                r_inst = nc.sync.dma_start(
                                    Lp[:],
                                    AP(s, 127 + 512 * c, [[W - 1, 128], [1, 512]]),
                                )
                                add_dep_helper(
                                    r_inst.ins,
                                    w_inst.ins,
                                    reason="shear read after strip write",
                                )
                                if c == c_diag:
                                    nc.gpsimd.affine_select(
                                        Lp[:],
                                        Lp[:],
                                        compare_op=mybir.AluOpType.is_ge,
                                        fill=-1e30,
                                        base=nb - 512 * c,
                                        pattern=[[-1, 512]],
                                        channel_multiplier=1,
                                    )
                                Lc = L_sb[:, 512 * c : 512 * (c + 1)]
                                nc.vector.tensor_add(Lc, c_ps[:], Lp[:])
                                nc.vector.reduce_max(
                                    out=cmax[:, c : c + 1], in_=Lc, axis=AX
                                )
                            negmax = work.tile([128, 1], f32, tag="negmax")
                            nc.vector.tensor_reduce(
                                out=negmax[:],
                                in_=cmax[:, :chunks],
                                axis=AX,
                                op=MAX,
                                negate=True,
                            )

                            # --- exp (+sumexp), normalize, transpose, AV ---
                            sume_c = work.tile([128, 4], f32, tag="sume")
                            P_all = lpool.tile([128, 2048], f16, tag="Pall")
                            for c in range(chunks):
                                nc.scalar.activation(
                                    P_all[:, 512 * c : 512 * (c + 1)],
                                    L_sb[:, 512 * c : 512 * (c + 1)],
                                    EXP,
                                    bias=negmax[:],
                                    scale=1.0,
                                    accum_out=sume_c[:, c : c + 1],
                                )
                            ssum = work.tile([128, 1], f32, tag="ssum")
                            nc.vector.tensor_reduce(
                                out=ssum[:],
                                in_=sume_c[:, :chunks],
                                axis=AX,
                                op=mybir.AluOpType.add,
                            )
                            rsum = work.tile([128, 1], f32, tag="rsum")
                            nc.vector.reciprocal(rsum[:], ssum[:])
                            at_ps = pa.tile([64, 128], f32, tag="avp")
                            for c in range(chunks):
                                nsub = min(4, i - 4 * c + 1)
                                nc.vector.tensor_scalar_mul(
                                    P_all[:, 512 * c : 512 * c + 128 * nsub],
                                    P_all[:, 512 * c : 512 * c + 128 * nsub],
                                    rsum[:],
                                )
                                for sb_ in range(nsub):
                                    mt = 4 * c + sb_
                                    pt_ps = pa.tile([128, 128], f16, tag="ptp")
                                    nc.tensor.transpose(
                                        pt_ps[:],
                                        P_all[:, 128 * mt : 128 * (mt + 1)],
                                        ident16[:],
                                    )
                                    pt_sb = work.tile([128, 128], f16, tag="pts")
                                    nc.vector.tensor_copy(pt_sb[:], pt_ps[:])
                                    nc.tensor.matmul(
                                        at_ps[:],
                                        V_all[:, z * 16 + mt, hs],
                                        pt_sb[:],
                                        start=(mt == 0),
                                        stop=(mt == i),
                                    )
                            nc.vector.tensor_copy(
                                attn16[hs, z * N + nb : z * N + nb + 128], at_ps[:]
                            )
                            if dbg and z == 0 and h == 0 and i == 15:
                                nc.sync.dma_start(dbg_t["d_L15"].ap(), L_sb[:])
                                nc.sync.dma_start(dbg_t["d_P15"].ap(), P_all[:])

            def phase_out(pa):
                if dbg:
                    nc.sync.dma_start(dbg_t["d_qT"].ap(), qT_all[:])
                    nc.sync.dma_start(dbg_t["d_kT"].ap(), kT_all[:])
                    nc.sync.dma_start(dbg_t["d_tabT"].ap(), tabT[:])
                    nc.sync.dma_start(
                        dbg_t["d_V"].ap(),
                        V_all[:].rearrange("p a b -> p (a b)"),
                    )
                    nc.sync.dma_start(dbg_t["d_attn16"].ap(), attn16[:])
                for z in range(B):
                    for dc in range(8):
                        for nn in range(4):
                            o_ps = pa.tile([128, 512], f32, tag="cps")
                            nc.tensor.matmul(
                                o_ps[:],
                                wo_sb[:, 128 * dc : 128 * (dc + 1)],
                                attn16[:, z * N + 512 * nn : z * N + 512 * (nn + 1)],
                                start=True,
                                stop=True,
                            )
                            o_sb = work.tile([128, 512], f32, tag="osb")
                            nc.scalar.copy(o_sb[:], o_ps[:])
                            nc.sync.dma_start(
                                outT.ap()[
                                    z,
                                    128 * dc : 128 * (dc + 1),
                                    512 * nn : 512 * (nn + 1),
                                ],
                                o_sb[:],
                            )

            def loop_body():
                phase_proj()
                with tc.tile_pool(name="pa", bufs=2, space="PSUM") as pa:
                    phase_attn(pa)
                    phase_out(pa)

            if loop == 1:
                loop_body()
            else:
                with tc.For_i(0, loop, 1):
                    loop_body()

    nc.compile()
    return nc


def _prep_inputs(x_q, x_kv, to_q, to_kv, for_pos_enc, to_o):
    xqT = np.ascontiguousarray(
        np.asarray(x_q, dtype=np.float32).transpose(2, 1, 0).reshape(D, NZ)
    )
    xkvT = np.ascontiguousarray(
        np.asarray(x_kv, dtype=np.float32).transpose(2, 1, 0).reshape(D, NZ)
    )
    xkv16 = xkvT.astype(np.float16)
    if "sctr" not in _cache:
        _cache["sctr"] = _sincos_rev()
    sctr = _cache["sctr"]
    to_q = np.asarray(to_q, dtype=np.float32)
    to_kv = np.asarray(to_kv, dtype=np.float32)
    fpe = np.asarray(for_pos_enc, dtype=np.float32)
    to_o = np.asarray(to_o, dtype=np.float32)
    in_maps = []
    for c in range(NCORES):
        hs = slice(HPC * c, HPC * (c + 1))
        wq = np.ascontiguousarray(to_q[hs].reshape(HPC * DQK, D).T)
        wk = np.ascontiguousarray(to_kv[hs, :DQK].reshape(HPC * DQK, D).T)
        wv = np.ascontiguousarray(to_kv[hs, DQK:].reshape(HPC * DV, D).T).astype(
            np.float16
        )
        wp = np.ascontiguousarray(fpe[hs].reshape(HPC * DQK, D).T)
        wo = np.ascontiguousarray(to_o[:, hs, :].reshape(D, HPC * DV).T).astype(
            np.float16
        )
        in_maps.append(
            {
                "xqT": xqT,
                "xkvT": xkvT,
                "xkv16": xkv16,
                "sctr": sctr,
                "wqT": wq,
                "wkT": wk,
                "wvT": wv,
                "wpT": wp,
                "woT": wo,
            }
        )
    return in_maps


def kernel(x_q, x_kv, to_q, to_kv, for_pos_enc, to_o):
    from concourse.bass_utils import run_bass_kernel_spmd

    if "nc" not in _cache:
        _cache["nc"] = _build()
    nc = _cache["nc"]
    in_maps = _prep_inputs(x_q, x_kv, to_q, to_kv, for_pos_enc, to_o)
    res = run_bass_kernel_spmd(nc, in_maps, core_ids=list(range(NCORES)))
    acc = np.zeros((B, D, N), dtype=np.float64)
    for c in range(NCORES):
        acc += res.results[c]["outT"].astype(np.float64)
    return np.ascontiguousarray(acc.transpose(2, 0, 1)).astype(np.float32)

